# revision 6
# baseline (speedup 1.0000x reference)
"""Trainium2 8-core kernel for the paired contrastive (NT-Xent-like) loss.

Math (tau=0.5, N=8192, D=256):
    z1 = l2norm(H_1), z2 = l2norm(H_2)
    den1_i = sum_j exp(z1.z1/t) + sum_j exp(z1.z2/t) - exp(|z1_i|^2/t)
    den2_i = sum_j exp(z2.z2/t) + sum_j exp(z2.z1/t) - exp(|z2_i|^2/t)
    loss = (1/2N) * sum_i [ ln(den1_i) + ln(den2_i) - 2*(z1_i.z2_i)/t ]
with |z_i|^2 == 1 analytically, so the subtracted diagonal is exp(1/t) = e^2.

Sharding: rows are split across 8 cores (1024 rows each).  Every core gets the
full (transposed, bf16) embedding matrices as the matmul "moving" operand and
its own row-block as the stationary operand, computes row-sums of
exp(S/t) for its four block-vs-all similarity streams, and emits one partial
scalar.  The host sums the 8 partials and divides by 2N.  No collectives.
"""

import math

import numpy as np
import ml_dtypes

import concourse.bass as bass
import concourse.tile as tile
from concourse import bacc, mybir
from concourse.bass_utils import run_bass_kernel_spmd

F32 = mybir.dt.float32
BF16 = mybir.dt.bfloat16
AF = mybir.ActivationFunctionType
ALU = mybir.AluOpType
AX = mybir.AxisListType

TAU = 0.5
E2 = math.exp(1.0 / TAU)  # analytic diag of the "refl" exp-similarity

N_FULL, D_FULL, N_CORES = 8192, 256, 8


def build_nc(N=N_FULL, D=D_FULL, n_cores=N_CORES):
    """Build the SPMD graph for one core (same graph runs on all cores)."""
    R = N // n_cores           # rows owned per core
    NK = D // 128              # contraction k-tiles
    CH = 512                   # column chunk (one PSUM bank of f32)
    NCH = N // CH              # chunks across full column dim
    CHB = min(CH, R)           # block column chunk width
    NCHB = (R + CH - 1) // CH  # chunks across block rows
    G = min(2048, N)           # exp-group width (<=4 PSUM banks)
    NG = N // G
    GCH = G // CH
    NRT = R // 128             # 128-row tiles per core
    L = 2 * N + 2 * R          # flat norm-vector length

    assert R % 128 == 0 and D % 128 == 0 and N % CH == 0 and N % G == 0

    nc = bacc.Bacc("TRN2", target_bir_lowering=False, debug=False,
                   num_devices=n_cores)

    ht = [nc.dram_tensor("HT1", [D, N], BF16, kind="ExternalInput"),
          nc.dram_tensor("HT2", [D, N], BF16, kind="ExternalInput")]
    hb = [nc.dram_tensor("Hb1", [D, R], BF16, kind="ExternalInput"),
          nc.dram_tensor("Hb2", [D, R], BF16, kind="ExternalInput")]
    out = nc.dram_tensor("out", [1, 1], F32, kind="ExternalOutput")

    with tile.TileContext(nc) as tc, tc.tile_pool(name="persist", bufs=1) as per:
        with (
            tc.tile_pool(name="work", bufs=4) as work,
            tc.tile_pool(name="prep_ps", bufs=2, space="PSUM") as pps,
        ):
            # --- persistent SBUF tensors -------------------------------
            Z = [[per.tile([128, N], BF16, tag=f"z{t}{k}", name=f"z{t}{k}")
                  for k in range(NK)] for t in range(2)]
            Zb = [[per.tile([128, R], BF16, tag=f"zb{t}{k}", name=f"zb{t}{k}")
                   for k in range(NK)] for t in range(2)]
            rows = {st: per.tile([128, NRT], F32, tag=f"rows_{st}",
                                 name=f"rows_{st}")
                    for st in ("s11", "s12", "s21", "s22")}
            # flat per-column 1/|h| vector (partition 0); filled with 1/|h|^2
            # then turned into 1/|h| by one in-place sqrt
            rq = per.tile([1, L], BF16, tag="rq", name="rq")
            ii_tot = per.tile([1, 1], F32, tag="ii_tot", name="ii_tot")
            lnacc = per.tile([128, 1], F32, tag="lnacc", name="lnacc")
            ones_k = per.tile([128, 1], BF16, tag="ones_k", name="ones_k")
            ones_b = per.tile([1, 128], BF16, tag="ones_b", name="ones_b")
            ones_f = per.tile([128, 1], F32, tag="ones_f", name="ones_f")
            zb = per.tile([128, 1], F32, tag="zb", name="zb")

            nc.gpsimd.memset(ones_k[:], 1.0)
            nc.gpsimd.memset(ones_b[:], 1.0)
            nc.gpsimd.memset(ones_f[:], 1.0)
            nc.gpsimd.memset(zb[:], 0.0)
            nc.gpsimd.memset(rq[:], 1.0)  # keep unused lanes sqrt-legal

            # --- prep: load + per-column 1/|h|^2 -----------------------
            def load_and_norm(dst, src, ncols, nch, chw, base):
                for c in range(nch):
                    w = min(chw, ncols - c * chw)
                    cs = slice(c * chw, c * chw + w)
                    pn = None
                    for k in range(NK):
                        nc.sync.dma_start(dst[k][:, cs], src[bass.ts(k, 128), cs])
                        sq = work.tile([128, CH], BF16, tag="sq", name="sq")
                        nc.vector.tensor_mul(sq[:, :w], dst[k][:, cs], dst[k][:, cs])
                        if k == 0:
                            pn = pps.tile([1, CH], F32, tag="pnorm", name="pnorm")
                        nc.tensor.matmul(pn[:, :w], ones_k[:], sq[:, :w],
                                         start=(k == 0), stop=(k == NK - 1))
                    st = work.tile([1, CH], F32, tag="rstage", name="rstage")
                    nc.vector.reciprocal(st[:, :w], pn[:, :w])
                    fo = slice(base + c * chw, base + c * chw + w)
                    nc.vector.tensor_copy(rq[0:1, fo], st[:, :w])

            load_and_norm(Z[0], ht[0].ap(), N, NCH, CH, 0)
            load_and_norm(Z[1], ht[1].ap(), N, NCH, CH, N)
            load_and_norm(Zb[0], hb[0].ap(), R, NCHB, CH, 2 * N)
            load_and_norm(Zb[1], hb[1].ap(), R, NCHB, CH, 2 * N + R)

            # one batched sqrt: rq = 1/|h| (bf16, partition 0)
            nc.scalar.activation(rq[:], rq[:], AF.Sqrt, bias=zb[:1, :])

            # --- scale columns: Z *= rinv (broadcast over partitions) --
            def scale(dst, ncols, nch, chw, base):
                for c in range(nch):
                    w = min(chw, ncols - c * chw)
                    cs = slice(c * chw, c * chw + w)
                    fo = slice(base + c * chw, base + c * chw + w)
                    pb = pps.tile([128, CH], F32, tag="pbcast", name="pbcast")
                    nc.tensor.matmul(pb[:, :w], ones_b[:], rq[0:1, fo],
                                     start=True, stop=True)
                    bb = work.tile([128, CH], BF16, tag="bb", name="bb")
                    nc.vector.tensor_copy(bb[:, :w], pb[:, :w])
                    for k in range(NK):
                        nc.vector.tensor_mul(dst[k][:, cs], dst[k][:, cs], bb[:, :w])

            scale(Z[0], N, NCH, CH, 0)
            scale(Z[1], N, NCH, CH, N)
            scale(Zb[0], R, NCHB, CH, 2 * N)
            scale(Zb[1], R, NCHB, CH, 2 * N + R)

            # --- S12 diagonal: sum_i z1_i . z2_i over own rows ---------
            prods = []
            for k in range(NK):
                pr = work.tile([128, R], BF16, tag=f"prod{k}", name=f"prod{k}")
                nc.vector.tensor_mul(pr[:], Zb[0][k][:], Zb[1][k][:])
                prods.append(pr)
            for c in range(NCHB):
                w = min(CHB, R - c * CHB)
                pii = pps.tile([1, CH], F32, tag="pnorm", name="pnorm")
                for k in range(NK):
                    nc.tensor.matmul(pii[:, :w], ones_k[:],
                                     prods[k][:, c * CHB:c * CHB + w],
                                     start=(k == 0), stop=(k == NK - 1))
                red = work.tile([1, 1], F32, tag="iired", name="iired")
                nc.vector.tensor_reduce(red[:], pii[:, :w], AX.X, ALU.add)
                if c == 0:
                    nc.vector.tensor_copy(ii_tot[:], red[:])
                else:
                    nc.vector.tensor_add(ii_tot[:], ii_tot[:], red[:])

        # --- four exp/row-sum streams ---------------------------------
        streams = [("s12", Zb[0], Z[1]), ("s21", Zb[1], Z[0]),
                   ("s11", Zb[0], Z[0]), ("s22", Zb[1], Z[1])]
        with (
            tc.tile_pool(name="spool", bufs=2, space="PSUM") as spool,
            tc.tile_pool(name="acc", bufs=3) as accp,
            tc.tile_pool(name="escp", bufs=3) as escp,
        ):
            for st, LHS, RHS in streams:
                for rt in range(NRT):
                    acc = accp.tile([128, NG], F32, tag="acc", name="acc")
                    for g in range(NG):
                        sg = spool.tile([128, G], F32, tag="sg", name="sg")
                        for gc in range(GCH):
                            o = slice(gc * CH, (gc + 1) * CH)
                            col = slice(g * G + gc * CH, g * G + (gc + 1) * CH)
                            for k in range(NK):
                                nc.tensor.matmul(sg[:, o], LHS[k][:, bass.ts(rt, 128)],
                                                 RHS[k][:, col],
                                                 start=(k == 0), stop=(k == NK - 1))
                        esc = escp.tile([128, G], BF16, tag="esc", name="esc")
                        nc.scalar.activation(esc[:], sg[:], AF.Exp, bias=zb[:],
                                             scale=1.0 / TAU,
                                             accum_out=acc[:, g:g + 1])
                    nc.vector.tensor_reduce(rows[st][:, rt:rt + 1], acc[:], AX.X, ALU.add)

        # --- final: ln(den1*den2) summed, minus (2/tau)*sum(diag) -----
        with (
            tc.tile_pool(name="fin", bufs=1) as fin,
            tc.tile_pool(name="fin_ps", bufs=1, space="PSUM") as fps,
        ):
            den1 = fin.tile([128, NRT], F32, tag="den1", name="den1")
            den2 = fin.tile([128, NRT], F32, tag="den2", name="den2")
            nc.vector.tensor_add(den1[:], rows["s11"][:], rows["s12"][:])
            nc.vector.tensor_scalar_add(den1[:], den1[:], -E2)
            nc.vector.tensor_add(den2[:], rows["s22"][:], rows["s21"][:])
            nc.vector.tensor_scalar_add(den2[:], den2[:], -E2)
            dd = fin.tile([128, NRT], F32, tag="dd", name="dd")
            nc.vector.tensor_mul(dd[:], den1[:], den2[:])
            lnout = fin.tile([128, NRT], F32, tag="lnout", name="lnout")
            nc.scalar.activation(lnout[:], dd[:], AF.Ln, bias=zb[:],
                                 accum_out=lnacc[:])
            # fold -(2/tau)*sum(S12_ii) into partition 0 of lnacc
            iim = fin.tile([1, 1], F32, tag="iim", name="iim")
            nc.vector.tensor_scalar_mul(iim[:], ii_tot[:], -2.0 / TAU)
            nc.vector.tensor_add(lnacc[0:1, :], lnacc[0:1, :], iim[:])
            ptot = fps.tile([1, 1], F32, tag="ptot", name="ptot")
            nc.tensor.matmul(ptot[:], ones_f[:], lnacc[:], start=True, stop=True)
            res = fin.tile([1, 1], F32, tag="res", name="res")
            nc.vector.tensor_copy(res[:], ptot[:])
            nc.sync.dma_start(out.ap()[:, :], res[:])

    nc.compile()
    return nc


_CACHE = {}


def _compiled(N=N_FULL, D=D_FULL, n_cores=N_CORES):
    key = (N, D, n_cores)
    if key not in _CACHE:
        _CACHE[key] = build_nc(N, D, n_cores)
    return _CACHE[key]


def make_in_maps(H_1, H_2, n_cores=N_CORES):
    H1 = np.asarray(H_1, dtype=np.float32)
    H2 = np.asarray(H_2, dtype=np.float32)
    N = H1.shape[0]
    R = N // n_cores
    HT1 = np.ascontiguousarray(H1.astype(ml_dtypes.bfloat16).T)
    HT2 = np.ascontiguousarray(H2.astype(ml_dtypes.bfloat16).T)
    maps = []
    for c in range(n_cores):
        sl = slice(c * R, (c + 1) * R)
        maps.append({
            "HT1": HT1, "HT2": HT2,
            "Hb1": np.ascontiguousarray(HT1[:, sl]),
            "Hb2": np.ascontiguousarray(HT2[:, sl]),
        })
    return maps


def kernel(H_1, H_2):
    N, D = H_1.shape
    nc = _compiled(N, D, N_CORES)
    in_maps = make_in_maps(H_1, H_2, N_CORES)
    res = run_bass_kernel_spmd(nc, in_maps, core_ids=list(range(N_CORES)))
    total = sum(float(r["out"][0, 0]) for r in res.results)
    return np.float32(total / (2.0 * N))


# revision 8
# speedup vs baseline: 1.2094x; 1.2094x over previous
"""Trainium2 8-core kernel for the paired contrastive (NT-Xent-like) loss.

Math (tau=0.5, N=8192, D=256):
    z1 = l2norm(H_1), z2 = l2norm(H_2)
    den1_i = sum_j exp(z1.z1/t) + sum_j exp(z1.z2/t) - exp(|z1_i|^2/t)
    den2_i = sum_j exp(z2.z2/t) + sum_j exp(z2.z1/t) - exp(|z2_i|^2/t)
    loss = (1/2N) * sum_i [ ln(den1_i) + ln(den2_i) - 2*(z1_i.z2_i)/t ]
with |z_i|^2 == 1 analytically, so the subtracted diagonal is exp(1/t) = e^2.

Sharding: rows split across 8 cores (1024 each); every core holds the full
(transposed, bf16) embeddings as the moving matmul operand.  Three exp
streams per core (S12, S11, S22); the S21 row-sums come from column-sums of
exp(S12/t) via a cross-core ReduceScatter (S21 = S12^T).  Each core emits one
partial scalar; the host sums them and divides by 2N.
"""

import math

import numpy as np
import ml_dtypes

import concourse.bass as bass
import concourse.bass_isa as bass_isa
import concourse.tile as tile
from concourse import bacc, mybir
from concourse.bass_utils import run_bass_kernel_spmd

F32 = mybir.dt.float32
BF16 = mybir.dt.bfloat16
AF = mybir.ActivationFunctionType
ALU = mybir.AluOpType
AX = mybir.AxisListType

TAU = 0.5
E2 = math.exp(1.0 / TAU)  # analytic diag of the "refl" exp-similarity

N_FULL, D_FULL, N_CORES = 8192, 256, 8


def build_nc(N=N_FULL, D=D_FULL, n_cores=N_CORES):
    """Build the SPMD graph for one core (same graph runs on all cores)."""
    R = N // n_cores           # rows owned per core
    NK = D // 128              # contraction k-tiles
    CH = 512                   # column chunk (one PSUM bank of f32)
    NCH = N // CH              # chunks across full column dim
    CHB = min(CH, R)           # block column chunk width
    NCHB = (R + CH - 1) // CH  # chunks across block rows
    G = min(2048, N)           # exp-group width (4 PSUM banks)
    NG = N // G
    GCH = G // CH
    NRT = R // 128             # 128-row tiles per core
    L = 2 * N + 2 * R          # flat norm-vector length
    SS = 2 * NCH + 2 * NCHB    # stacked norm rows

    assert R % 128 == 0 and D % 128 == 0 and N % CH == 0 and N % G == 0
    assert SS <= 128

    nc = bacc.Bacc("TRN2", target_bir_lowering=False, debug=False,
                   num_devices=n_cores)

    ht = [nc.dram_tensor("HT1", [D, N], BF16, kind="ExternalInput"),
          nc.dram_tensor("HT2", [D, N], BF16, kind="ExternalInput")]
    hb = [nc.dram_tensor("Hb1", [D, R], BF16, kind="ExternalInput"),
          nc.dram_tensor("Hb2", [D, R], BF16, kind="ExternalInput")]
    out = nc.dram_tensor("out", [1, 1], F32, kind="ExternalOutput")

    with tile.TileContext(nc) as tc, \
         tc.tile_pool(name="persist", bufs=1) as per, \
         tc.tile_pool(name="dram", bufs=1, space="DRAM") as dram:
        # --- persistent tensors ---------------------------------------
        Z = [[per.tile([128, N], BF16, tag=f"z{t}{k}", name=f"z{t}{k}")
              for k in range(NK)] for t in range(2)]
        Zb = [[per.tile([128, R], BF16, tag=f"zb{t}{k}", name=f"zb{t}{k}")
               for k in range(NK)] for t in range(2)]
        rows = {st: per.tile([128, NRT], F32, tag=f"rows_{st}",
                             name=f"rows_{st}")
                for st in ("s11", "s12", "s22")}
        colacc = per.tile([128, N], F32, tag="colacc", name="colacc")
        dn = per.tile([128, NRT], F32, tag="dn", name="dn")
        ssk = per.tile([SS, CH], F32, tag="ssk", name="ssk")
        rvk = per.tile([SS, CH], BF16, tag="rvk", name="rvk")
        rq = per.tile([1, L], BF16, tag="rq", name="rq")
        ii_tot = per.tile([1, 1], F32, tag="ii_tot", name="ii_tot")
        lnacc = per.tile([128, 1], F32, tag="lnacc", name="lnacc")
        ones_k = per.tile([128, 1], BF16, tag="ones_k", name="ones_k")
        ones_b = per.tile([1, 128], BF16, tag="ones_b", name="ones_b")
        ones_f = per.tile([128, 1], F32, tag="ones_f", name="ones_f")
        zb = per.tile([128, 1], F32, tag="zb", name="zb")
        cc_in = dram.tile([N], F32, tag="cc_in", name="cc_in")
        cc_out = dram.tile([R], F32, tag="cc_out", name="cc_out")

        nc.gpsimd.memset(ones_k[:], 1.0)
        nc.gpsimd.memset(ones_b[:], 1.0)
        nc.gpsimd.memset(ones_f[:], 1.0)
        nc.gpsimd.memset(zb[:], 0.0)
        nc.gpsimd.memset(ssk[:], 1.0)  # unused lanes must stay recip/sqrt-legal

        specs = [  # (dst tiles, dram src, ncols, nchunks, ss-row base, rq base)
            (Z[0], ht[0].ap(), N, NCH, 0, 0),
            (Z[1], ht[1].ap(), N, NCH, NCH, N),
            (Zb[0], hb[0].ap(), R, NCHB, 2 * NCH, 2 * N),
            (Zb[1], hb[1].ap(), R, NCHB, 2 * NCH + NCHB, 2 * N + R),
        ]

        with tc.tile_pool(name="work", bufs=4) as work, \
             tc.tile_pool(name="prep_ps", bufs=2, space="PSUM") as pps:
            # --- load + per-column |h|^2 into stacked rows -------------
            for dst, src, ncols, nch, srow, _ in specs:
                for c in range(nch):
                    w = min(CH, ncols - c * CH)
                    cs = slice(c * CH, c * CH + w)
                    pn = None
                    for k in range(NK):
                        nc.sync.dma_start(dst[k][:, cs], src[bass.ts(k, 128), cs])
                        sq = work.tile([128, CH], BF16, tag="sq", name="sq")
                        nc.vector.tensor_mul(sq[:, :w], dst[k][:, cs], dst[k][:, cs])
                        if k == 0:
                            pn = pps.tile([1, CH], F32, tag="pnorm", name="pnorm")
                        nc.tensor.matmul(pn[:, :w], ones_k[:], sq[:, :w],
                                         start=(k == 0), stop=(k == NK - 1))
                    stg = work.tile([1, CH], F32, tag="stg", name="stg")
                    nc.vector.tensor_copy(stg[:, :w], pn[:, :w])
                    r = srow + c
                    nc.sync.dma_start(ssk[r:r + 1, :w], stg[:, :w])

            # one batched rinv = sqrt(1/|h|^2), cast bf16, scatter flat
            nc.vector.reciprocal(ssk[:], ssk[:])
            nc.scalar.activation(ssk[:], ssk[:], AF.Sqrt, bias=zb[:SS, :])
            nc.vector.tensor_copy(rvk[:], ssk[:])
            for dst, src, ncols, nch, srow, rbase in specs:
                for c in range(nch):
                    w = min(CH, ncols - c * CH)
                    r = srow + c
                    nc.sync.dma_start(rq[0:1, rbase + c * CH:rbase + c * CH + w],
                                      rvk[r:r + 1, :w])

            # --- scale columns: Z *= rinv (broadcast via K=1 matmul) ---
            for dst, src, ncols, nch, srow, rbase in specs:
                for c in range(nch):
                    w = min(CH, ncols - c * CH)
                    cs = slice(c * CH, c * CH + w)
                    fo = slice(rbase + c * CH, rbase + c * CH + w)
                    pb = pps.tile([128, CH], F32, tag="pbcast", name="pbcast")
                    nc.tensor.matmul(pb[:, :w], ones_b[:], rq[0:1, fo],
                                     start=True, stop=True)
                    bb = work.tile([128, CH], BF16, tag="bb", name="bb")
                    nc.vector.tensor_copy(bb[:, :w], pb[:, :w])
                    for k in range(NK):
                        nc.vector.tensor_mul(dst[k][:, cs], dst[k][:, cs],
                                             bb[:, :w])

            # --- S12 diagonal: sum_i z1_i . z2_i over own rows ---------
            prods = []
            for k in range(NK):
                pr = work.tile([128, R], BF16, tag=f"prod{k}", name=f"prod{k}")
                nc.vector.tensor_mul(pr[:], Zb[0][k][:], Zb[1][k][:])
                prods.append(pr)
            for c in range(NCHB):
                w = min(CHB, R - c * CHB)
                pii = pps.tile([1, CH], F32, tag="pnorm", name="pnorm")
                for k in range(NK):
                    nc.tensor.matmul(pii[:, :w], ones_k[:],
                                     prods[k][:, c * CHB:c * CHB + w],
                                     start=(k == 0), stop=(k == NK - 1))
                red = work.tile([1, 1], F32, tag="iired", name="iired")
                nc.vector.tensor_reduce(red[:], pii[:, :w], AX.X, ALU.add)
                if c == 0:
                    nc.vector.tensor_copy(ii_tot[:], red[:])
                else:
                    nc.vector.tensor_add(ii_tot[:], ii_tot[:], red[:])

        # --- exp/row-sum streams (S21 row-sums = S12 col-sums) --------
        streams = [("s12", Zb[0], Z[1]), ("s11", Zb[0], Z[0]),
                   ("s22", Zb[1], Z[1])]
        with (
            tc.tile_pool(name="spool", bufs=2, space="PSUM") as spool,
            tc.tile_pool(name="acc", bufs=3) as accp,
            tc.tile_pool(name="escp", bufs=3) as escp,
        ):
            for st, LHS, RHS in streams:
                for rt in range(NRT):
                    acc = accp.tile([128, NG], F32, tag="acc", name="acc")
                    for g in range(NG):
                        sg = spool.tile([128, G], F32, tag="sg", name="sg")
                        for k in range(NK):
                            for gc in range(GCH):
                                o = slice(gc * CH, (gc + 1) * CH)
                                col = slice(g * G + gc * CH,
                                            g * G + (gc + 1) * CH)
                                nc.tensor.matmul(sg[:, o],
                                                 LHS[k][:, bass.ts(rt, 128)],
                                                 RHS[k][:, col],
                                                 start=(k == 0),
                                                 stop=(k == NK - 1))
                        esc = escp.tile([128, G], BF16, tag="esc", name="esc")
                        nc.scalar.activation(esc[:], sg[:], AF.Exp, bias=zb[:],
                                             scale=1.0 / TAU,
                                             accum_out=acc[:, g:g + 1])
                        if st == "s12":
                            gs = slice(g * G, (g + 1) * G)
                            if rt == 0:
                                nc.vector.tensor_copy(colacc[:, gs], esc[:])
                            else:
                                nc.vector.tensor_add(colacc[:, gs],
                                                     colacc[:, gs], esc[:])
                    nc.vector.tensor_reduce(rows[st][:, rt:rt + 1], acc[:],
                                            AX.X, ALU.add)
                if st == "s12":
                    # S12 col-sums: reduce own 1024 rows, then sum row-blocks
                    # across cores; ReduceScatter hands each core its rows.
                    nc.gpsimd.partition_all_reduce(colacc[:], colacc[:], 128,
                                                   bass_isa.ReduceOp.add)
                    nc.sync.dma_start(cc_in[:], colacc[0:1, :])
                    nc.gpsimd.collective_compute(
                        "ReduceScatter", ALU.add,
                        replica_groups=[list(range(n_cores))],
                        ins=[cc_in.opt()], outs=[cc_out.opt()])
                    nc.sync.dma_start(dn[:],
                                      cc_out.rearrange("(t p) -> p t", p=128))

        # --- final: ln(den1*den2) summed, minus (2/tau)*sum(diag) -----
        with (
            tc.tile_pool(name="fin", bufs=1) as fin,
            tc.tile_pool(name="fin_ps", bufs=1, space="PSUM") as fps,
        ):
            den1 = fin.tile([128, NRT], F32, tag="den1", name="den1")
            den2 = fin.tile([128, NRT], F32, tag="den2", name="den2")
            nc.vector.tensor_add(den1[:], rows["s11"][:], rows["s12"][:])
            nc.vector.tensor_scalar_add(den1[:], den1[:], -E2)
            nc.vector.tensor_add(den2[:], rows["s22"][:], dn[:])
            nc.vector.tensor_scalar_add(den2[:], den2[:], -E2)
            dd = fin.tile([128, NRT], F32, tag="dd", name="dd")
            nc.vector.tensor_mul(dd[:], den1[:], den2[:])
            lnout = fin.tile([128, NRT], F32, tag="lnout", name="lnout")
            nc.scalar.activation(lnout[:], dd[:], AF.Ln, bias=zb[:],
                                 accum_out=lnacc[:])
            iim = fin.tile([1, 1], F32, tag="iim", name="iim")
            nc.vector.tensor_scalar_mul(iim[:], ii_tot[:], -2.0 / TAU)
            nc.vector.tensor_add(lnacc[0:1, :], lnacc[0:1, :], iim[:])
            ptot = fps.tile([1, 1], F32, tag="ptot", name="ptot")
            nc.tensor.matmul(ptot[:], ones_f[:], lnacc[:], start=True, stop=True)
            res = fin.tile([1, 1], F32, tag="res", name="res")
            nc.vector.tensor_copy(res[:], ptot[:])
            nc.sync.dma_start(out.ap()[:, :], res[:])

    nc.compile()
    return nc


_CACHE = {}


def _compiled(N=N_FULL, D=D_FULL, n_cores=N_CORES):
    key = (N, D, n_cores)
    if key not in _CACHE:
        _CACHE[key] = build_nc(N, D, n_cores)
    return _CACHE[key]


def make_in_maps(H_1, H_2, n_cores=N_CORES):
    H1 = np.asarray(H_1, dtype=np.float32)
    H2 = np.asarray(H_2, dtype=np.float32)
    N = H1.shape[0]
    R = N // n_cores
    HT1 = np.ascontiguousarray(H1.astype(ml_dtypes.bfloat16).T)
    HT2 = np.ascontiguousarray(H2.astype(ml_dtypes.bfloat16).T)
    maps = []
    for c in range(n_cores):
        sl = slice(c * R, (c + 1) * R)
        maps.append({
            "HT1": HT1, "HT2": HT2,
            "Hb1": np.ascontiguousarray(HT1[:, sl]),
            "Hb2": np.ascontiguousarray(HT2[:, sl]),
        })
    return maps


def kernel(H_1, H_2):
    N, D = H_1.shape
    nc = _compiled(N, D, N_CORES)
    in_maps = make_in_maps(H_1, H_2, N_CORES)
    res = run_bass_kernel_spmd(nc, in_maps, core_ids=list(range(N_CORES)))
    total = sum(float(r["out"][0, 0]) for r in res.results)
    return np.float32(total / (2.0 * N))


# revision 12
# speedup vs baseline: 1.4044x; 1.1612x over previous
"""Trainium2 8-core kernel for the paired contrastive (NT-Xent-like) loss.

Math (tau=0.5, N=8192, D=256):
    z1 = l2norm(H_1), z2 = l2norm(H_2)
    den1_i = sum_j exp(z1.z1/t) + sum_j exp(z1.z2/t) - exp(|z1_i|^2/t)
    den2_i = sum_j exp(z2.z2/t) + sum_j exp(z2.z1/t) - exp(|z2_i|^2/t)
    loss = (1/2N) * sum_i [ ln(den1_i) + ln(den2_i) - 2*(z1_i.z2_i)/t ]
with |z_i|^2 == 1 analytically, so the subtracted diagonal is exp(1/t) = e^2.

Sharding: rows split across 8 cores (1024 each); every core holds the full
(transposed, bf16) embeddings as the moving matmul operand.  Three exp
streams per core (S12, S11, S22); the S21 row-sums come from column-sums of
exp(S12/t) via a cross-core ReduceScatter (S21 = S12^T).  Each core emits one
partial scalar; the host sums them and divides by 2N.
"""

import math

import numpy as np
import ml_dtypes

import concourse.bass as bass
import concourse.bass_isa as bass_isa
import concourse.tile as tile
from concourse import bacc, mybir
from concourse.bass_utils import run_bass_kernel_spmd

F32 = mybir.dt.float32
BF16 = mybir.dt.bfloat16
AF = mybir.ActivationFunctionType
ALU = mybir.AluOpType
AX = mybir.AxisListType

TAU = 0.5
E2 = math.exp(1.0 / TAU)  # analytic diag of the "refl" exp-similarity

N_FULL, D_FULL, N_CORES = 8192, 256, 8


def build_nc(N=N_FULL, D=D_FULL, n_cores=N_CORES):
    """Build the SPMD graph for one core (same graph runs on all cores)."""
    R = N // n_cores           # rows owned per core
    NK = D // 128              # contraction k-tiles
    CH = 512                   # column chunk (one PSUM bank of f32)
    NCH = N // CH              # chunks across full column dim
    CHB = min(CH, R)           # block column chunk width
    NCHB = (R + CH - 1) // CH  # chunks across block rows
    G = min(2048, N)           # exp-group width (4 PSUM banks)
    NG = N // G
    GCH = G // CH
    NRT = R // 128             # 128-row tiles per core
    L = 2 * N + 2 * R          # flat norm-vector length
    SS = 2 * NCH + 2 * NCHB    # stacked norm rows

    assert R % 128 == 0 and D % 128 == 0 and N % CH == 0 and N % G == 0
    assert SS <= 128

    nc = bacc.Bacc("TRN2", target_bir_lowering=False, debug=False,
                   num_devices=n_cores)

    ht = [nc.dram_tensor("HT1", [D, N], BF16, kind="ExternalInput"),
          nc.dram_tensor("HT2", [D, N], BF16, kind="ExternalInput")]
    hb = [nc.dram_tensor("Hb1", [D, R], BF16, kind="ExternalInput"),
          nc.dram_tensor("Hb2", [D, R], BF16, kind="ExternalInput")]
    out = nc.dram_tensor("out", [1, 1], F32, kind="ExternalOutput")

    with tile.TileContext(nc) as tc, \
         tc.tile_pool(name="persist", bufs=1) as per, \
         tc.tile_pool(name="dram", bufs=1, space="DRAM") as dram:
        # --- persistent tensors ---------------------------------------
        Z = [[per.tile([128, N], BF16, tag=f"z{t}{k}", name=f"z{t}{k}")
              for k in range(NK)] for t in range(2)]
        Zb = [[per.tile([128, R], BF16, tag=f"zb{t}{k}", name=f"zb{t}{k}")
               for k in range(NK)] for t in range(2)]
        rows = {st: per.tile([128, NRT], F32, tag=f"rows_{st}",
                             name=f"rows_{st}")
                for st in ("s11", "s12", "s22")}
        colacc = per.tile([128, N], F32, tag="colacc", name="colacc")
        dn = per.tile([128, NRT], F32, tag="dn", name="dn")
        ssk = per.tile([SS, CH], F32, tag="ssk", name="ssk")
        rvk = per.tile([SS, CH], BF16, tag="rvk", name="rvk")
        ii_tot = per.tile([1, 1], F32, tag="ii_tot", name="ii_tot")
        lnacc = per.tile([128, 1], F32, tag="lnacc", name="lnacc")
        ones_k = per.tile([128, 1], BF16, tag="ones_k", name="ones_k")
        ones_b = per.tile([1, 128], BF16, tag="ones_b", name="ones_b")
        ones_f = per.tile([128, 1], F32, tag="ones_f", name="ones_f")
        zb = per.tile([128, 1], F32, tag="zb", name="zb")
        cc_in = dram.tile([N], F32, tag="cc_in", name="cc_in")
        cc_out = dram.tile([R], F32, tag="cc_out", name="cc_out")

        nc.gpsimd.memset(ones_k[:], 1.0)
        nc.gpsimd.memset(ones_b[:], 1.0)
        nc.gpsimd.memset(ones_f[:], 1.0)
        nc.gpsimd.memset(zb[:], 0.0)
        nc.gpsimd.memset(ssk[:], 1.0)  # unused lanes must stay recip/sqrt-legal

        # order: Z[1], Zb[0], Zb[1] first so streams s12/s22 can start while
        # Z[0] (only needed by s11, last stream) is still being prepared
        specs = [  # (dst tiles, dram src, ncols, nchunks, ss-row base, dma engine)
            (Z[1], ht[1].ap(), N, NCH, NCH, nc.sync),
            (Zb[0], hb[0].ap(), R, NCHB, 2 * NCH, nc.sync),
            (Zb[1], hb[1].ap(), R, NCHB, 2 * NCH + NCHB, nc.sync),
            (Z[0], ht[0].ap(), N, NCH, 0, nc.scalar),
        ]

        with tc.tile_pool(name="work", bufs=4) as work, \
             tc.tile_pool(name="bbp", bufs=1) as bbp, \
             tc.tile_pool(name="prep_ps", bufs=2, space="PSUM") as pps:
            # --- load (one big DMA per k-tile) + per-column |h|^2 ------
            for dst, src, ncols, nch, srow, eng in specs:
                for k in range(NK):
                    eng.dma_start(dst[k][:], src[bass.ts(k, 128), :])
                for c in range(nch):
                    w = min(CH, ncols - c * CH)
                    cs = slice(c * CH, c * CH + w)
                    pn = pps.tile([1, CH], F32, tag="pnorm", name="pnorm")
                    for k in range(NK):
                        sq = work.tile([128, CH], BF16, tag="sq", name="sq")
                        nc.vector.tensor_mul(sq[:, :w], dst[k][:, cs],
                                             dst[k][:, cs])
                        nc.tensor.matmul(pn[:, :w], ones_k[:], sq[:, :w],
                                         start=(k == 0), stop=(k == NK - 1))
                    stg = work.tile([1, CH], F32, tag="stg", name="stg")
                    nc.vector.tensor_copy(stg[:, :w], pn[:, :w])
                    r = srow + c
                    nc.gpsimd.dma_start(ssk[r:r + 1, :w], stg[:, :w])

            # one batched rinv = sqrt(1/|h|^2), cast bf16
            nc.vector.reciprocal(ssk[:], ssk[:])
            nc.scalar.activation(ssk[:], ssk[:], AF.Sqrt, bias=zb[:SS, :])
            nc.vector.tensor_copy(rvk[:], ssk[:])

            # --- scale columns: Z *= rinv (broadcast via K=1 matmul,
            #     psum->bf16 copies on the idle ACT engine) --------------
            for dst, src, ncols, nch, srow, eng in specs:
                bb = bbp.tile([128, ncols], BF16, tag=f"bb{ncols}", name="bb")
                for c in range(nch):
                    w = min(CH, ncols - c * CH)
                    cs = slice(c * CH, c * CH + w)
                    r = srow + c
                    rst = work.tile([1, CH], BF16, tag="rst", name="rst")
                    nc.gpsimd.dma_start(rst[:, :w], rvk[r:r + 1, :w])
                    pb = pps.tile([128, CH], F32, tag="pbcast", name="pbcast")
                    nc.tensor.matmul(pb[:, :w], ones_b[:], rst[0:1, :w],
                                     start=True, stop=True)
                    nc.scalar.activation(bb[:, cs], pb[:, :w], AF.Copy)
                for k in range(NK):
                    nc.vector.tensor_mul(dst[k][:], dst[k][:], bb[:])

            # --- S12 diagonal: sum_i z1_i . z2_i over own rows ---------
            prods = []
            for k in range(NK):
                pr = work.tile([128, R], BF16, tag=f"prod{k}", name=f"prod{k}")
                nc.vector.tensor_mul(pr[:], Zb[0][k][:], Zb[1][k][:])
                prods.append(pr)
            for c in range(NCHB):
                w = min(CHB, R - c * CHB)
                pii = pps.tile([1, CH], F32, tag="pnorm", name="pnorm")
                for k in range(NK):
                    nc.tensor.matmul(pii[:, :w], ones_k[:],
                                     prods[k][:, c * CHB:c * CHB + w],
                                     start=(k == 0), stop=(k == NK - 1))
                red = work.tile([1, 1], F32, tag="iired", name="iired")
                nc.vector.tensor_reduce(red[:], pii[:, :w], AX.X, ALU.add)
                if c == 0:
                    nc.vector.tensor_copy(ii_tot[:], red[:])
                else:
                    nc.vector.tensor_add(ii_tot[:], ii_tot[:], red[:])

        # --- exp/row-sum streams (S21 row-sums = S12 col-sums) --------
        streams = [("s12", Zb[0], Z[1]), ("s22", Zb[1], Z[1]),
                   ("s11", Zb[0], Z[0])]
        with (
            tc.tile_pool(name="spool", bufs=2, space="PSUM") as spool,
            tc.tile_pool(name="acc", bufs=3) as accp,
            tc.tile_pool(name="escp", bufs=3) as escp,
        ):
            for st, LHS, RHS in streams:
                for rt in range(NRT):
                    acc = accp.tile([128, NG], F32, tag="acc", name="acc")
                    for g in range(NG):
                        sg = spool.tile([128, G], F32, tag="sg", name="sg")
                        for k in range(NK):
                            for gc in range(GCH):
                                o = slice(gc * CH, (gc + 1) * CH)
                                col = slice(g * G + gc * CH,
                                            g * G + (gc + 1) * CH)
                                nc.tensor.matmul(sg[:, o],
                                                 LHS[k][:, bass.ts(rt, 128)],
                                                 RHS[k][:, col],
                                                 start=(k == 0),
                                                 stop=(k == NK - 1))
                        esc = escp.tile([128, G], BF16, tag="esc", name="esc")
                        nc.scalar.activation(esc[:], sg[:], AF.Exp, bias=zb[:],
                                             scale=1.0 / TAU,
                                             accum_out=acc[:, g:g + 1])
                        if st == "s12":
                            gs = slice(g * G, (g + 1) * G)
                            if rt == 0:
                                nc.vector.tensor_copy(colacc[:, gs], esc[:])
                            else:
                                nc.vector.tensor_add(colacc[:, gs],
                                                     colacc[:, gs], esc[:])
                    nc.vector.tensor_reduce(rows[st][:, rt:rt + 1], acc[:],
                                            AX.X, ALU.add)
                if st == "s12":
                    # S12 col-sums: reduce own 1024 rows, then sum row-blocks
                    # across cores; ReduceScatter hands each core its rows.
                    nc.gpsimd.partition_all_reduce(colacc[:], colacc[:], 128,
                                                   bass_isa.ReduceOp.add)
                    nc.sync.dma_start(cc_in[:], colacc[0:1, :])
                    nc.gpsimd.collective_compute(
                        "ReduceScatter", ALU.add,
                        replica_groups=[list(range(n_cores))],
                        ins=[cc_in.opt()], outs=[cc_out.opt()])
                    nc.sync.dma_start(dn[:],
                                      cc_out.rearrange("(t p) -> p t", p=128))

        # --- final: ln(den1*den2) summed, minus (2/tau)*sum(diag) -----
        with (
            tc.tile_pool(name="fin", bufs=1) as fin,
            tc.tile_pool(name="fin_ps", bufs=1, space="PSUM") as fps,
        ):
            den1 = fin.tile([128, NRT], F32, tag="den1", name="den1")
            den2 = fin.tile([128, NRT], F32, tag="den2", name="den2")
            nc.vector.tensor_add(den1[:], rows["s11"][:], rows["s12"][:])
            nc.vector.tensor_scalar_add(den1[:], den1[:], -E2)
            nc.vector.tensor_add(den2[:], rows["s22"][:], dn[:])
            nc.vector.tensor_scalar_add(den2[:], den2[:], -E2)
            dd = fin.tile([128, NRT], F32, tag="dd", name="dd")
            nc.vector.tensor_mul(dd[:], den1[:], den2[:])
            lnout = fin.tile([128, NRT], F32, tag="lnout", name="lnout")
            nc.scalar.activation(lnout[:], dd[:], AF.Ln, bias=zb[:],
                                 accum_out=lnacc[:])
            iim = fin.tile([1, 1], F32, tag="iim", name="iim")
            nc.vector.tensor_scalar_mul(iim[:], ii_tot[:], -2.0 / TAU)
            nc.vector.tensor_add(lnacc[0:1, :], lnacc[0:1, :], iim[:])
            ptot = fps.tile([1, 1], F32, tag="ptot", name="ptot")
            nc.tensor.matmul(ptot[:], ones_f[:], lnacc[:], start=True, stop=True)
            res = fin.tile([1, 1], F32, tag="res", name="res")
            nc.vector.tensor_copy(res[:], ptot[:])
            nc.sync.dma_start(out.ap()[:, :], res[:])

    nc.compile()
    return nc


_CACHE = {}


def _compiled(N=N_FULL, D=D_FULL, n_cores=N_CORES):
    key = (N, D, n_cores)
    if key not in _CACHE:
        _CACHE[key] = build_nc(N, D, n_cores)
    return _CACHE[key]


def make_in_maps(H_1, H_2, n_cores=N_CORES):
    H1 = np.asarray(H_1, dtype=np.float32)
    H2 = np.asarray(H_2, dtype=np.float32)
    N = H1.shape[0]
    R = N // n_cores
    HT1 = np.ascontiguousarray(H1.astype(ml_dtypes.bfloat16).T)
    HT2 = np.ascontiguousarray(H2.astype(ml_dtypes.bfloat16).T)
    maps = []
    for c in range(n_cores):
        sl = slice(c * R, (c + 1) * R)
        maps.append({
            "HT1": HT1, "HT2": HT2,
            "Hb1": np.ascontiguousarray(HT1[:, sl]),
            "Hb2": np.ascontiguousarray(HT2[:, sl]),
        })
    return maps


def kernel(H_1, H_2):
    N, D = H_1.shape
    nc = _compiled(N, D, N_CORES)
    in_maps = make_in_maps(H_1, H_2, N_CORES)
    res = run_bass_kernel_spmd(nc, in_maps, core_ids=list(range(N_CORES)))
    total = sum(float(r["out"][0, 0]) for r in res.results)
    return np.float32(total / (2.0 * N))
